# revision 6
# baseline (speedup 1.0000x reference)
"""GroupQueryAttention TRN2 Bass kernel, v2.

Problem: B=4, T=2048, C=1024, H=16 heads, G=4 groups, head_dim=64, causal.
Sharding: 8 cores = 4 batches (DP) x 2 tensor-parallel halves (8 heads /
2 groups each). Host pre-transposes x and weight slices to bf16; each core
computes a partial output projection over its 512 attention channels; host
sums the two TP partials per batch and adds the bias.

v2 design (vs v1 at ~329us):
- all-bf16 datapath (halves DMA bytes; bf16 matmuls stream 1 col/cycle and
  get FWL, fp32r measured ~1.5 cyc/col)
- head pairs (2p4, 2p4+1) share one [128,2,512] psum score tile (2 banks)
  so ONE 1024-col exp ACTIVATE serves both heads: the ACT engine is the
  bottleneck (139k exp columns + ~300 cyc/instruction overhead) and halving
  the instruction count cuts its overhead in half
- skew-1 software pipelining in the attention inner loop: scores for step t
  are emitted before PV for step t-1, so the PE never head-of-line blocks
  on the ACT exp (v1 lost ~30% PE occupancy to this)
- score matmul pairs are emitted back-to-back on row bands 0:64 / 64:128
  (tile_position row groups) so the PE can overlap them
- ~20 warmup matmuls on a memset tile at t=0: HAM clock-gate releases only
  after ~3.4us of sustained PE activity (v1 ran its first 53us at 1.2 GHz)
- phase fusion: projections for block j+2 and output projections for
  blocks j-2/j-1 are interleaved into attention block j's steps, keeping
  the ACT exp stream and the PE both busy end-to-end
"""

import sys
import numpy as np
import ml_dtypes

for _p in ("/opt/trn_rl_repo", "/opt/trn_rl_repo/concourse"):
    if _p not in sys.path:
        sys.path.insert(0, _p)

import concourse.bass as bass  # noqa: E402
import concourse.mybir as mybir  # noqa: E402
from concourse import bacc  # noqa: E402
from concourse.tile import TileContext  # noqa: E402
from concourse.bass_utils import run_bass_kernel_spmd  # noqa: E402
from concourse.masks import make_identity, make_upper_triangular  # noqa: E402

F32 = mybir.dt.float32
BF16 = mybir.dt.bfloat16
EXP = mybir.ActivationFunctionType.Exp

B, T, C = 4, 2048, 1024
NH, NG, HD = 16, 4, 64
NH_LOC, NG_LOC = 8, 2          # per-core heads / groups
S = NH_LOC * HD                # 512 local attention channels
TQB = 512                      # tq block
NTQB = T // TQB                # 4
NCT = C // 128                 # 8 contraction tiles
SCALE = float(HD) ** -0.5
N_WARMUP = 22


def _build_program():
    nc = bacc.Bacc("TRN2", target_bir_lowering=False, debug=False, num_devices=8)

    xT = nc.dram_tensor("xT", [C, T], BF16, kind="ExternalInput")
    wqT = nc.dram_tensor("wqT", [C, S], BF16, kind="ExternalInput")
    wkT = nc.dram_tensor("wkT", [C, NG_LOC * HD], BF16, kind="ExternalInput")
    wvT = nc.dram_tensor("wvT", [C, NG_LOC * HD], BF16, kind="ExternalInput")
    wpT = nc.dram_tensor("wpT", [S, C], BF16, kind="ExternalInput")
    y = nc.dram_tensor("y", [T, C], F32, kind="ExternalOutput")

    with TileContext(nc) as tc:
        with tc.tile_pool(name="const", bufs=1) as const_pool, \
             tc.tile_pool(name="persist", bufs=1) as persist, \
             tc.tile_pool(name="vtp", bufs=2) as vtp, \
             tc.tile_pool(name="pp", bufs=3) as ppool, \
             tc.tile_pool(name="attn", bufs=4) as apool, \
             tc.tile_pool(name="sm", bufs=2) as small, \
             tc.tile_pool(name="yo", bufs=2) as ypool, \
             tc.tile_pool(name="psS", bufs=2, space="PSUM") as psS, \
             tc.tile_pool(name="psO", bufs=1, space="PSUM") as psO, \
             tc.tile_pool(name="psM", bufs=1, space="PSUM") as psM, \
             tc.tile_pool(name="psT", bufs=1, space="PSUM") as psT:

            # ---- warmup first: PE busy from t~0 releases the HAM gate ----
            wsrc = const_pool.tile([128, 512], F32)
            nc.vector.memset(wsrc, 0.125)
            wtile = const_pool.tile([128, 512], BF16)
            nc.vector.tensor_copy(wtile, wsrc)
            for _ in range(N_WARMUP):
                pswu = psM.tile([128, 512], F32, tag="mm", name="pswu")
                nc.tensor.matmul(pswu, wtile[:, 0:128], wtile,
                                 start=True, stop=True)

            # ---- constants ----
            ident = const_pool.tile([128, 64], F32)
            make_identity(nc, ident[0:64, 0:64])
            make_identity(nc, ident[64:128, 0:64], nomemset=False)
            mask32 = const_pool.tile([128, 128], F32)
            make_upper_triangular(nc, mask32, val=1.0, diag=True)
            mask = const_pool.tile([128, 128], BF16)
            nc.vector.tensor_copy(mask, mask32)
            # ---- persistent SBUF ----
            qt = [persist.tile([128, T], BF16, tag=f"qt{i}", name=f"qt{i}")
                  for i in range(4)]
            kdup = [persist.tile([128, T], BF16, tag=f"kd{g}", name=f"kd{g}")
                    for g in range(NG_LOC)]
            v_sb = [persist.tile([128, T], BF16, tag=f"v{g}", name=f"v{g}")
                    for g in range(NG_LOC)]
            xts = [persist.tile([128, T], BF16, tag=f"x{ct}", name=f"x{ct}")
                   for ct in range(NCT)]
            wq_sb = [persist.tile([128, S], BF16, tag=f"wq{ct}", name=f"wq{ct}")
                     for ct in range(NCT)]
            wk_sb = [persist.tile([128, 128], BF16, tag=f"wk{ct}", name=f"wk{ct}")
                     for ct in range(NCT)]
            wv_sb = [persist.tile([128, 128], BF16, tag=f"wv{ct}", name=f"wv{ct}")
                     for ct in range(NCT)]
            wp_sb = [persist.tile([128, C], BF16, tag=f"wp{i}", name=f"wp{i}")
                     for i in range(4)]

            # ---- DMAs: first x halves, then weights, then second x halves ----
            for ct in range(NCT):
                nc.sync.dma_start(out=xts[ct][:, 0:1024],
                                  in_=xT[ct * 128:(ct + 1) * 128, 0:1024])
            for ct in range(NCT):
                nc.sync.dma_start(out=wq_sb[ct], in_=wqT[ct * 128:(ct + 1) * 128, :])
                nc.sync.dma_start(out=wk_sb[ct], in_=wkT[ct * 128:(ct + 1) * 128, :])
                nc.sync.dma_start(out=wv_sb[ct], in_=wvT[ct * 128:(ct + 1) * 128, :])
            for ct in range(NCT):
                nc.sync.dma_start(out=xts[ct][:, 1024:2048],
                                  in_=xT[ct * 128:(ct + 1) * 128, 1024:2048])
            for i in range(4):
                nc.sync.dma_start(out=wp_sb[i], in_=wpT[i * 128:(i + 1) * 128, :])

            # ones columns of v_sb (denominator trick)
            ones64 = const_pool.tile([128, 64], F32)
            nc.vector.memset(ones64, 1.0)
            for g in range(NG_LOC):
                for t in range(T // 128):
                    nc.vector.tensor_copy(
                        v_sb[g][:, t * 128:t * 128 + 64], ones64)

            # ---- generators for interleavable PE work ----
            def proj_block(j, use_s_pool):
                """Projections q/k/v for tq/tk block j + v transpose."""
                cols = slice(j * TQB, (j + 1) * TQB)

                def fresh():
                    if use_s_pool:
                        psx = psS.tile([128, 2 * TQB], F32, tag="s", name="psx")
                        return psx[:, 0:TQB]
                    return psM.tile([128, TQB], F32, tag="mm", name="psm")

                for p4 in range(4):
                    dst = fresh()
                    for ct in range(NCT):
                        nc.tensor.matmul(
                            dst, wq_sb[ct][:, p4 * 128:(p4 + 1) * 128],
                            xts[ct][:, cols], start=(ct == 0), stop=(ct == NCT - 1))
                        yield
                    nc.vector.tensor_copy(qt[p4][:, cols], dst)
                # k (both groups in one psum: g0 on 0:64, g1 on 64:128)
                dst = fresh()
                for ct in range(NCT):
                    nc.tensor.matmul(dst, wk_sb[ct], xts[ct][:, cols],
                                     start=(ct == 0), stop=(ct == NCT - 1))
                    yield
                for g in range(NG_LOC):
                    rows = slice(g * 64, (g + 1) * 64)
                    nc.vector.tensor_copy(kdup[g][0:64, cols], dst[rows, :])
                    nc.vector.tensor_copy(kdup[g][64:128, cols], dst[rows, :])
                # v -> vt (sbuf) -> per-128-block transpose into v_sb
                dst = fresh()
                for ct in range(NCT):
                    nc.tensor.matmul(dst, wv_sb[ct], xts[ct][:, cols],
                                     start=(ct == 0), stop=(ct == NCT - 1))
                    yield
                vt = vtp.tile([128, TQB], F32, tag="vt", name="vt")
                nc.vector.tensor_copy(vt, dst)
                for g in range(NG_LOC):
                    for ts_ in range(4):
                        t_abs = 4 * j + ts_
                        pst = psT.tile([128, 512], F32, tag="tr", name="pst")
                        nc.tensor.transpose(
                            pst[:, 0:64],
                            vt[g * 64:(g + 1) * 64, ts_ * 128:(ts_ + 1) * 128],
                            ident[g * 64:(g + 1) * 64, 0:64])
                        yield
                        nc.vector.tensor_copy(
                            v_sb[g][:, t_abs * 128 + 64:(t_abs + 1) * 128],
                            pst[:, 0:64])

            def outproj_block(j, at_tiles):
                """Output projection for tq block j (4 tau rows of 128)."""
                for tt in range(4):
                    tau = 4 * j + tt
                    ysb = ypool.tile([128, C], F32, tag="y", name="ysb")
                    for half in range(2):
                        if (tt * 2 + half) % 2 == 0:
                            yp = psM.tile([128, TQB], F32, tag="mm", name="yp")
                        else:
                            yp = psT.tile([128, TQB], F32, tag="tr", name="yp")
                        for p4 in range(4):
                            nc.tensor.matmul(
                                yp, at_tiles[p4][:, tt * 128:(tt + 1) * 128],
                                wp_sb[p4][:, half * TQB:(half + 1) * TQB],
                                start=(p4 == 0), stop=(p4 == 3))
                            yield
                        nc.vector.tensor_copy(
                            ysb[:, half * TQB:(half + 1) * TQB], yp)
                    nc.sync.dma_start(
                        out=y[tau * 128:(tau + 1) * 128, :], in_=ysb)

            # ---- attention ----
            def emit_ep(j, p4, t, ps, po, ntk, rcp_tile=None):
                """exp + mask + PV pair for step t of pair p4, block j."""
                g = p4 // 2
                c = t - 4 * j
                off = max(0, c * 128)
                pt = ppool.tile([128, 2 * TQB], BF16, tag="pt", name="pt")
                if off == 0:
                    # both heads' regions are contiguous: one 1024-col exp
                    nc.scalar.activation(pt[:, :], ps[:, :], EXP, scale=SCALE)
                else:
                    nc.scalar.activation(pt[:, off:TQB], ps[:, off:TQB],
                                         EXP, scale=SCALE)
                    nc.scalar.activation(pt[:, TQB + off:2 * TQB],
                                         ps[:, TQB + off:2 * TQB],
                                         EXP, scale=SCALE)
                if c >= 0:
                    nc.gpsimd.tensor_mul(
                        pt[:, off:off + 128], pt[:, off:off + 128], mask)
                    nc.gpsimd.tensor_mul(
                        pt[:, TQB + off:TQB + off + 128],
                        pt[:, TQB + off:TQB + off + 128], mask)
                for h01 in range(2):
                    nc.tensor.matmul(
                        po[:, h01 * TQB + off:(h01 + 1) * TQB],
                        v_sb[g][:, t * 128:(t + 1) * 128],
                        pt[:, h01 * TQB + off:(h01 + 1) * TQB],
                        start=(t == 0), stop=(t == ntk - 1))
                    if t == ntk - 1 and rcp_tile is not None:
                        nc.vector.reciprocal_approx_fast(
                            rcp_tile[0:64, h01 * TQB:(h01 + 1) * TQB],
                            po[0:64, h01 * TQB:(h01 + 1) * TQB])

            def attention_block(j, feed, rate):
                """Attention for tq block j; drains `feed` generators at
                ~`rate` PE ops per step."""
                tq0 = j * TQB
                ntk = 4 * (j + 1)
                at_tiles = [apool.tile([128, TQB], BF16, tag=f"at{p4}",
                                       name=f"at{j}_{p4}")
                            for p4 in range(4)]
                budget = 0.0
                for p4 in range(4):
                    g = p4 // 2
                    po = psO.tile([128, 2 * TQB], F32, tag="po", name="po")
                    ps_prev = None
                    for t in range(ntk):
                        c = t - 4 * j
                        off = max(0, c * 128)
                        ps = psS.tile([128, 2 * TQB], F32, tag="s", name="ps")
                        nc.tensor.matmul(
                            ps[:, off:TQB],
                            kdup[g][0:64, t * 128:(t + 1) * 128],
                            qt[p4][0:64, tq0 + off:tq0 + TQB],
                            start=True, stop=True)
                        nc.tensor.matmul(
                            ps[:, TQB + off:2 * TQB],
                            kdup[g][64:128, t * 128:(t + 1) * 128],
                            qt[p4][64:128, tq0 + off:tq0 + TQB],
                            start=True, stop=True)
                        if t > 0:
                            emit_ep(j, p4, t - 1, ps_prev, po, ntk)
                        ps_prev = ps
                        budget += rate
                        while budget >= 1.0 and feed:
                            try:
                                next(feed[0])
                                budget -= 1.0
                            except StopIteration:
                                feed.pop(0)
                    rcp = small.tile([128, 2 * TQB], F32, tag="rcp", name="rcp")
                    emit_ep(j, p4, ntk - 1, ps_prev, po, ntk, rcp_tile=rcp)
                    # normalization (recips were emitted inside emit_ep)
                    nc.vector.tensor_mul(
                        at_tiles[p4][0:64, :], po[64:128, 0:TQB],
                        rcp[0:64, 0:TQB])
                    nc.vector.tensor_mul(
                        at_tiles[p4][64:128, :], po[64:128, TQB:2 * TQB],
                        rcp[0:64, TQB:2 * TQB])
                return at_tiles

            def drain(gen):
                for _ in gen:
                    pass

            # ---- schedule ----
            drain(proj_block(0, use_s_pool=True))
            drain(proj_block(1, use_s_pool=True))
            feed = [proj_block(2, use_s_pool=False),
                    proj_block(3, use_s_pool=False)]
            at0 = attention_block(0, feed, 1.35)
            feed.append(outproj_block(0, at0))
            at1 = attention_block(1, feed, 1.35)
            feed.append(outproj_block(1, at1))
            at2 = attention_block(2, feed, 1.25)
            feed.append(outproj_block(2, at2))
            at3 = attention_block(3, feed, 1.2)
            for gen in feed:
                drain(gen)
            drain(outproj_block(3, at3))

    nc.compile()
    return nc


_NC_CACHE = None


def _get_nc():
    global _NC_CACHE
    if _NC_CACHE is None:
        _NC_CACHE = _build_program()
    return _NC_CACHE


def _bf16(a):
    return np.ascontiguousarray(a).astype(ml_dtypes.bfloat16)


def _make_in_maps(x, Wq, Wk, Wv, Wp):
    in_maps = []
    for core in range(8):
        b, tp = core // 2, core % 2
        hs = slice(tp * NH_LOC, (tp + 1) * NH_LOC)
        gs = slice(tp * NG_LOC, (tp + 1) * NG_LOC)
        in_maps.append({
            "xT": _bf16(x[b].T),
            "wqT": _bf16(Wq[hs].transpose(2, 0, 1).reshape(C, S)),
            "wkT": _bf16(Wk[gs].transpose(2, 0, 1).reshape(C, NG_LOC * HD)),
            "wvT": _bf16(Wv[gs].transpose(2, 0, 1).reshape(C, NG_LOC * HD)),
            "wpT": _bf16(Wp[:, tp * S:(tp + 1) * S].T),
        })
    return in_maps


def kernel(x, Wq, Wk, Wv, Wp, bp, _trace=False):
    x = np.asarray(x, dtype=np.float32)
    nc = _get_nc()
    in_maps = _make_in_maps(
        x, np.asarray(Wq, np.float32), np.asarray(Wk, np.float32),
        np.asarray(Wv, np.float32), np.asarray(Wp, np.float32))
    res = run_bass_kernel_spmd(nc, in_maps, list(range(8)), trace=_trace)
    out = np.empty((B, T, C), dtype=np.float32)
    bp32 = np.asarray(bp, np.float32)
    for b in range(B):
        out[b] = res.results[2 * b]["y"] + res.results[2 * b + 1]["y"] + bp32
    if _trace:
        return out, res
    return out


# revision 7
# speedup vs baseline: 1.0675x; 1.0675x over previous
"""GroupQueryAttention TRN2 Bass kernel, v2.

Problem: B=4, T=2048, C=1024, H=16 heads, G=4 groups, head_dim=64, causal.
Sharding: 8 cores = 4 batches (DP) x 2 tensor-parallel halves (8 heads /
2 groups each). Host pre-transposes x and weight slices to bf16; each core
computes a partial output projection over its 512 attention channels; host
sums the two TP partials per batch and adds the bias.

v2 design (vs v1 at ~329us):
- all-bf16 datapath (halves DMA bytes; bf16 matmuls stream 1 col/cycle and
  get FWL, fp32r measured ~1.5 cyc/col)
- head pairs (2p4, 2p4+1) share one [128,2,512] psum score tile (2 banks)
  so ONE 1024-col exp ACTIVATE serves both heads: the ACT engine is the
  bottleneck (139k exp columns + ~300 cyc/instruction overhead) and halving
  the instruction count cuts its overhead in half
- skew-1 software pipelining in the attention inner loop: scores for step t
  are emitted before PV for step t-1, so the PE never head-of-line blocks
  on the ACT exp (v1 lost ~30% PE occupancy to this)
- score matmul pairs are emitted back-to-back on row bands 0:64 / 64:128
  (tile_position row groups) so the PE can overlap them
- ~20 warmup matmuls on a memset tile at t=0: HAM clock-gate releases only
  after ~3.4us of sustained PE activity (v1 ran its first 53us at 1.2 GHz)
- phase fusion: projections for block j+2 and output projections for
  blocks j-2/j-1 are interleaved into attention block j's steps, keeping
  the ACT exp stream and the PE both busy end-to-end
"""

import sys
import numpy as np
import ml_dtypes

for _p in ("/opt/trn_rl_repo", "/opt/trn_rl_repo/concourse"):
    if _p not in sys.path:
        sys.path.insert(0, _p)

import concourse.bass as bass  # noqa: E402
import concourse.mybir as mybir  # noqa: E402
from concourse import bacc  # noqa: E402
from concourse.tile import TileContext  # noqa: E402
from concourse.bass_utils import run_bass_kernel_spmd  # noqa: E402
from concourse.masks import make_identity, make_upper_triangular  # noqa: E402

F32 = mybir.dt.float32
BF16 = mybir.dt.bfloat16
EXP = mybir.ActivationFunctionType.Exp

B, T, C = 4, 2048, 1024
NH, NG, HD = 16, 4, 64
NH_LOC, NG_LOC = 8, 2          # per-core heads / groups
S = NH_LOC * HD                # 512 local attention channels
TQB = 512                      # tq block
NTQB = T // TQB                # 4
NCT = C // 128                 # 8 contraction tiles
SCALE = float(HD) ** -0.5
N_WARMUP = 22


def _build_program():
    nc = bacc.Bacc("TRN2", target_bir_lowering=False, debug=False, num_devices=8)

    xT = nc.dram_tensor("xT", [C, T], BF16, kind="ExternalInput")
    wqT = nc.dram_tensor("wqT", [C, S], BF16, kind="ExternalInput")
    wkT = nc.dram_tensor("wkT", [C, NG_LOC * HD], BF16, kind="ExternalInput")
    wvT = nc.dram_tensor("wvT", [C, NG_LOC * HD], BF16, kind="ExternalInput")
    wpT = nc.dram_tensor("wpT", [S, C], BF16, kind="ExternalInput")
    y = nc.dram_tensor("y", [T, C], BF16, kind="ExternalOutput")

    with TileContext(nc) as tc:
        with tc.tile_pool(name="const", bufs=1) as const_pool, \
             tc.tile_pool(name="persist", bufs=1) as persist, \
             tc.tile_pool(name="vtp", bufs=2) as vtp, \
             tc.tile_pool(name="pp", bufs=3) as ppool, \
             tc.tile_pool(name="attn", bufs=4) as apool, \
             tc.tile_pool(name="sm", bufs=2) as small, \
             tc.tile_pool(name="yo", bufs=2) as ypool, \
             tc.tile_pool(name="psS", bufs=2, space="PSUM") as psS, \
             tc.tile_pool(name="psO", bufs=1, space="PSUM") as psO, \
             tc.tile_pool(name="psM", bufs=1, space="PSUM") as psM, \
             tc.tile_pool(name="psT", bufs=1, space="PSUM") as psT:

            # ---- warmup first: PE busy from t~0 releases the HAM gate ----
            wtile = const_pool.tile([128, 512], BF16)
            nc.gpsimd.memset(wtile, 0.125)
            for _ in range(N_WARMUP):
                pswu = psM.tile([128, 512], F32, tag="mm", name="pswu")
                nc.tensor.matmul(pswu, wtile[:, 0:128], wtile,
                                 start=True, stop=True)

            # ---- constants ----
            ident = const_pool.tile([128, 64], F32)
            make_identity(nc, ident[0:64, 0:64])
            make_identity(nc, ident[64:128, 0:64], nomemset=False)
            mask32 = const_pool.tile([128, 128], F32)
            make_upper_triangular(nc, mask32, val=1.0, diag=True)
            mask = const_pool.tile([128, 128], BF16)
            nc.vector.tensor_copy(mask, mask32)
            # ---- persistent SBUF ----
            qt = [persist.tile([128, T], BF16, tag=f"qt{i}", name=f"qt{i}")
                  for i in range(4)]
            kdup = [persist.tile([128, T], BF16, tag=f"kd{g}", name=f"kd{g}")
                    for g in range(NG_LOC)]
            v_sb = [persist.tile([128, T], BF16, tag=f"v{g}", name=f"v{g}")
                    for g in range(NG_LOC)]
            xts = [persist.tile([128, T], BF16, tag=f"x{ct}", name=f"x{ct}")
                   for ct in range(NCT)]
            wq_sb = [persist.tile([128, S], BF16, tag=f"wq{ct}", name=f"wq{ct}")
                     for ct in range(NCT)]
            wk_sb = [persist.tile([128, 128], BF16, tag=f"wk{ct}", name=f"wk{ct}")
                     for ct in range(NCT)]
            wv_sb = [persist.tile([128, 128], BF16, tag=f"wv{ct}", name=f"wv{ct}")
                     for ct in range(NCT)]
            wp_sb = [persist.tile([128, C], BF16, tag=f"wp{i}", name=f"wp{i}")
                     for i in range(4)]

            # ---- DMAs: first x halves, then weights, then second x halves ----
            for ct in range(NCT):
                nc.sync.dma_start(out=xts[ct][:, 0:1024],
                                  in_=xT[ct * 128:(ct + 1) * 128, 0:1024])
            for ct in range(NCT):
                nc.sync.dma_start(out=wq_sb[ct], in_=wqT[ct * 128:(ct + 1) * 128, :])
                nc.sync.dma_start(out=wk_sb[ct], in_=wkT[ct * 128:(ct + 1) * 128, :])
                nc.sync.dma_start(out=wv_sb[ct], in_=wvT[ct * 128:(ct + 1) * 128, :])
            for ct in range(NCT):
                nc.sync.dma_start(out=xts[ct][:, 1024:2048],
                                  in_=xT[ct * 128:(ct + 1) * 128, 1024:2048])
            for i in range(4):
                nc.sync.dma_start(out=wp_sb[i], in_=wpT[i * 128:(i + 1) * 128, :])

            # ones columns of v_sb (denominator trick)
            ones64 = const_pool.tile([128, 64], F32)
            nc.vector.memset(ones64, 1.0)
            for g in range(NG_LOC):
                for t in range(T // 128):
                    nc.vector.tensor_copy(
                        v_sb[g][:, t * 128:t * 128 + 64], ones64)

            # ---- generators for interleavable PE work ----
            def proj_block(j, use_s_pool):
                """Projections q/k/v for tq/tk block j + v transpose."""
                cols = slice(j * TQB, (j + 1) * TQB)

                def fresh():
                    if use_s_pool:
                        psx = psS.tile([128, 2 * TQB], F32, tag="s", name="psx")
                        return psx[:, 0:TQB]
                    return psM.tile([128, TQB], F32, tag="mm", name="psm")

                for p4 in range(4):
                    dst = fresh()
                    for ct in range(NCT):
                        nc.tensor.matmul(
                            dst, wq_sb[ct][:, p4 * 128:(p4 + 1) * 128],
                            xts[ct][:, cols], start=(ct == 0), stop=(ct == NCT - 1))
                        yield
                    nc.vector.tensor_copy(qt[p4][:, cols], dst)
                # k (both groups in one psum: g0 on 0:64, g1 on 64:128)
                dst = fresh()
                for ct in range(NCT):
                    nc.tensor.matmul(dst, wk_sb[ct], xts[ct][:, cols],
                                     start=(ct == 0), stop=(ct == NCT - 1))
                    yield
                for g in range(NG_LOC):
                    rows = slice(g * 64, (g + 1) * 64)
                    nc.vector.tensor_copy(kdup[g][0:64, cols], dst[rows, :])
                    nc.vector.tensor_copy(kdup[g][64:128, cols], dst[rows, :])
                # v -> vt (sbuf) -> per-128-block transpose into v_sb
                dst = fresh()
                for ct in range(NCT):
                    nc.tensor.matmul(dst, wv_sb[ct], xts[ct][:, cols],
                                     start=(ct == 0), stop=(ct == NCT - 1))
                    yield
                vt = vtp.tile([128, TQB], F32, tag="vt", name="vt")
                nc.vector.tensor_copy(vt, dst)
                for g in range(NG_LOC):
                    for ts_ in range(4):
                        t_abs = 4 * j + ts_
                        pst = psT.tile([128, 512], F32, tag="tr", name="pst")
                        nc.tensor.transpose(
                            pst[:, 0:64],
                            vt[g * 64:(g + 1) * 64, ts_ * 128:(ts_ + 1) * 128],
                            ident[g * 64:(g + 1) * 64, 0:64])
                        yield
                        nc.vector.tensor_copy(
                            v_sb[g][:, t_abs * 128 + 64:(t_abs + 1) * 128],
                            pst[:, 0:64])

            def outproj_block(j, at_tiles):
                """Output projection for tq block j (4 tau rows of 128)."""
                for tt in range(4):
                    tau = 4 * j + tt
                    ysb = ypool.tile([128, C], BF16, tag="y", name="ysb")
                    for half in range(2):
                        if (tt * 2 + half) % 2 == 0:
                            yp = psM.tile([128, TQB], F32, tag="mm", name="yp")
                        else:
                            yp = psT.tile([128, TQB], F32, tag="tr", name="yp")
                        for p4 in range(4):
                            nc.tensor.matmul(
                                yp, at_tiles[p4][:, tt * 128:(tt + 1) * 128],
                                wp_sb[p4][:, half * TQB:(half + 1) * TQB],
                                start=(p4 == 0), stop=(p4 == 3))
                            yield
                        cols = slice(half * TQB, (half + 1) * TQB)
                        nc.vector.tensor_copy(ysb[:, cols], yp)
                        nc.sync.dma_start(
                            out=y[tau * 128:(tau + 1) * 128, cols],
                            in_=ysb[:, cols])

            # ---- attention ----
            def emit_ep(j, p4, t, ps, po, ntk, rcp_tile=None):
                """exp + mask + PV pair for step t of pair p4, block j."""
                g = p4 // 2
                c = t - 4 * j
                off = max(0, c * 128)
                pt = ppool.tile([128, 2 * TQB], BF16, tag="pt", name="pt")
                if off == 0:
                    # both heads' regions are contiguous: one 1024-col exp
                    nc.scalar.activation(pt[:, :], ps[:, :], EXP, scale=SCALE)
                else:
                    nc.scalar.activation(pt[:, off:TQB], ps[:, off:TQB],
                                         EXP, scale=SCALE)
                    nc.scalar.activation(pt[:, TQB + off:2 * TQB],
                                         ps[:, TQB + off:2 * TQB],
                                         EXP, scale=SCALE)
                if c >= 0:
                    nc.gpsimd.tensor_mul(
                        pt[:, off:off + 128], pt[:, off:off + 128], mask)
                    nc.gpsimd.tensor_mul(
                        pt[:, TQB + off:TQB + off + 128],
                        pt[:, TQB + off:TQB + off + 128], mask)
                for h01 in range(2):
                    nc.tensor.matmul(
                        po[:, h01 * TQB + off:(h01 + 1) * TQB],
                        v_sb[g][:, t * 128:(t + 1) * 128],
                        pt[:, h01 * TQB + off:(h01 + 1) * TQB],
                        start=(t == 0), stop=(t == ntk - 1))
                    if t == ntk - 1 and rcp_tile is not None:
                        nc.vector.reciprocal_approx_fast(
                            rcp_tile[0:64, h01 * TQB:(h01 + 1) * TQB],
                            po[0:64, h01 * TQB:(h01 + 1) * TQB])

            def attention_block(j, feed, rate):
                """Attention for tq block j; drains `feed` generators at
                ~`rate` PE ops per step."""
                tq0 = j * TQB
                ntk = 4 * (j + 1)
                at_tiles = [apool.tile([128, TQB], BF16, tag=f"at{p4}",
                                       name=f"at{j}_{p4}")
                            for p4 in range(4)]
                budget = 0.0
                for p4 in range(4):
                    g = p4 // 2
                    po = psO.tile([128, 2 * TQB], F32, tag="po", name="po")
                    ps_prev = None
                    for t in range(ntk):
                        c = t - 4 * j
                        off = max(0, c * 128)
                        ps = psS.tile([128, 2 * TQB], F32, tag="s", name="ps")
                        nc.tensor.matmul(
                            ps[:, off:TQB],
                            kdup[g][0:64, t * 128:(t + 1) * 128],
                            qt[p4][0:64, tq0 + off:tq0 + TQB],
                            start=True, stop=True)
                        nc.tensor.matmul(
                            ps[:, TQB + off:2 * TQB],
                            kdup[g][64:128, t * 128:(t + 1) * 128],
                            qt[p4][64:128, tq0 + off:tq0 + TQB],
                            start=True, stop=True)
                        if t > 0:
                            emit_ep(j, p4, t - 1, ps_prev, po, ntk)
                        ps_prev = ps
                        budget += rate
                        while budget >= 1.0 and feed:
                            try:
                                next(feed[0])
                                budget -= 1.0
                            except StopIteration:
                                feed.pop(0)
                    rcp = small.tile([128, 2 * TQB], F32, tag="rcp", name="rcp")
                    emit_ep(j, p4, ntk - 1, ps_prev, po, ntk, rcp_tile=rcp)
                    # normalization (recips were emitted inside emit_ep)
                    nc.vector.tensor_mul(
                        at_tiles[p4][0:64, :], po[64:128, 0:TQB],
                        rcp[0:64, 0:TQB])
                    nc.vector.tensor_mul(
                        at_tiles[p4][64:128, :], po[64:128, TQB:2 * TQB],
                        rcp[0:64, TQB:2 * TQB])
                return at_tiles

            def drain(gen):
                for _ in gen:
                    pass

            # ---- schedule ----
            drain(proj_block(0, use_s_pool=True))
            feed = [proj_block(1, use_s_pool=False),
                    proj_block(2, use_s_pool=False),
                    proj_block(3, use_s_pool=False)]
            at0 = attention_block(0, feed, 3.8)
            feed.append(outproj_block(0, at0))
            at1 = attention_block(1, feed, 1.45)
            feed.append(outproj_block(1, at1))
            at2 = attention_block(2, feed, 1.3)
            feed.append(outproj_block(2, at2))
            at3 = attention_block(3, feed, 1.2)
            for gen in feed:
                drain(gen)
            drain(outproj_block(3, at3))

    nc.compile()
    return nc


_NC_CACHE = None


def _get_nc():
    global _NC_CACHE
    if _NC_CACHE is None:
        _NC_CACHE = _build_program()
    return _NC_CACHE


def _bf16(a):
    return np.ascontiguousarray(a).astype(ml_dtypes.bfloat16)


def _make_in_maps(x, Wq, Wk, Wv, Wp):
    in_maps = []
    for core in range(8):
        b, tp = core // 2, core % 2
        hs = slice(tp * NH_LOC, (tp + 1) * NH_LOC)
        gs = slice(tp * NG_LOC, (tp + 1) * NG_LOC)
        in_maps.append({
            "xT": _bf16(x[b].T),
            "wqT": _bf16(Wq[hs].transpose(2, 0, 1).reshape(C, S)),
            "wkT": _bf16(Wk[gs].transpose(2, 0, 1).reshape(C, NG_LOC * HD)),
            "wvT": _bf16(Wv[gs].transpose(2, 0, 1).reshape(C, NG_LOC * HD)),
            "wpT": _bf16(Wp[:, tp * S:(tp + 1) * S].T),
        })
    return in_maps


def kernel(x, Wq, Wk, Wv, Wp, bp, _trace=False):
    x = np.asarray(x, dtype=np.float32)
    nc = _get_nc()
    in_maps = _make_in_maps(
        x, np.asarray(Wq, np.float32), np.asarray(Wk, np.float32),
        np.asarray(Wv, np.float32), np.asarray(Wp, np.float32))
    res = run_bass_kernel_spmd(nc, in_maps, list(range(8)), trace=_trace)
    out = np.empty((B, T, C), dtype=np.float32)
    bp32 = np.asarray(bp, np.float32)
    for b in range(B):
        out[b] = (res.results[2 * b]["y"].astype(np.float32)
                  + res.results[2 * b + 1]["y"].astype(np.float32) + bp32)
    if _trace:
        return out, res
    return out


# revision 8
# speedup vs baseline: 1.0833x; 1.0148x over previous
"""GroupQueryAttention TRN2 Bass kernel, v2.

Problem: B=4, T=2048, C=1024, H=16 heads, G=4 groups, head_dim=64, causal.
Sharding: 8 cores = 4 batches (DP) x 2 tensor-parallel halves (8 heads /
2 groups each). Host pre-transposes x and weight slices to bf16; each core
computes a partial output projection over its 512 attention channels; host
sums the two TP partials per batch and adds the bias.

v2 design (vs v1 at ~329us):
- all-bf16 datapath (halves DMA bytes; bf16 matmuls stream 1 col/cycle and
  get FWL, fp32r measured ~1.5 cyc/col)
- head pairs (2p4, 2p4+1) share one [128,2,512] psum score tile (2 banks)
  so ONE 1024-col exp ACTIVATE serves both heads: the ACT engine is the
  bottleneck (139k exp columns + ~300 cyc/instruction overhead) and halving
  the instruction count cuts its overhead in half
- skew-1 software pipelining in the attention inner loop: scores for step t
  are emitted before PV for step t-1, so the PE never head-of-line blocks
  on the ACT exp (v1 lost ~30% PE occupancy to this)
- score matmul pairs are emitted back-to-back on row bands 0:64 / 64:128
  (tile_position row groups) so the PE can overlap them
- ~20 warmup matmuls on a memset tile at t=0: HAM clock-gate releases only
  after ~3.4us of sustained PE activity (v1 ran its first 53us at 1.2 GHz)
- phase fusion: projections for block j+2 and output projections for
  blocks j-2/j-1 are interleaved into attention block j's steps, keeping
  the ACT exp stream and the PE both busy end-to-end
"""

import sys
import numpy as np
import ml_dtypes

for _p in ("/opt/trn_rl_repo", "/opt/trn_rl_repo/concourse"):
    if _p not in sys.path:
        sys.path.insert(0, _p)

import concourse.bass as bass  # noqa: E402
import concourse.mybir as mybir  # noqa: E402
from concourse import bacc  # noqa: E402
from concourse.tile import TileContext  # noqa: E402
from concourse.bass_utils import run_bass_kernel_spmd  # noqa: E402
from concourse.masks import make_identity, make_upper_triangular  # noqa: E402

F32 = mybir.dt.float32
BF16 = mybir.dt.bfloat16
EXP = mybir.ActivationFunctionType.Exp

B, T, C = 4, 2048, 1024
NH, NG, HD = 16, 4, 64
NH_LOC, NG_LOC = 8, 2          # per-core heads / groups
S = NH_LOC * HD                # 512 local attention channels
TQB = 512                      # tq block
NTQB = T // TQB                # 4
NCT = C // 128                 # 8 contraction tiles
SCALE = float(HD) ** -0.5
N_WARMUP = 12


def _build_program():
    nc = bacc.Bacc("TRN2", target_bir_lowering=False, debug=False, num_devices=8)

    xT = nc.dram_tensor("xT", [C, T], BF16, kind="ExternalInput")
    wqT = nc.dram_tensor("wqT", [C, S], BF16, kind="ExternalInput")
    wkT = nc.dram_tensor("wkT", [C, NG_LOC * HD], BF16, kind="ExternalInput")
    wvT = nc.dram_tensor("wvT", [C, NG_LOC * HD], BF16, kind="ExternalInput")
    wpT = nc.dram_tensor("wpT", [S, C], BF16, kind="ExternalInput")
    y = nc.dram_tensor("y", [T, C], BF16, kind="ExternalOutput")

    with TileContext(nc) as tc:
        with tc.tile_pool(name="const", bufs=1) as const_pool, \
             tc.tile_pool(name="persist", bufs=1) as persist, \
             tc.tile_pool(name="vtp", bufs=2) as vtp, \
             tc.tile_pool(name="pp", bufs=3) as ppool, \
             tc.tile_pool(name="attn", bufs=4) as apool, \
             tc.tile_pool(name="sm", bufs=2) as small, \
             tc.tile_pool(name="yo", bufs=2) as ypool, \
             tc.tile_pool(name="psS", bufs=2, space="PSUM") as psS, \
             tc.tile_pool(name="psO", bufs=1, space="PSUM") as psO, \
             tc.tile_pool(name="psM", bufs=1, space="PSUM") as psM, \
             tc.tile_pool(name="psT", bufs=1, space="PSUM") as psT:

            # ---- warmup first: PE busy from t~0 releases the HAM gate ----
            wtile = const_pool.tile([128, 512], BF16)
            nc.gpsimd.memset(wtile, 0.125)
            for _ in range(N_WARMUP):
                pswu = psM.tile([128, 512], F32, tag="mm", name="pswu")
                nc.tensor.matmul(pswu, wtile[:, 0:128], wtile,
                                 start=True, stop=True)

            # ---- constants ----
            ident = const_pool.tile([128, 64], F32)
            make_identity(nc, ident[0:64, 0:64])
            make_identity(nc, ident[64:128, 0:64], nomemset=False)
            mask32 = const_pool.tile([128, 128], F32)
            make_upper_triangular(nc, mask32, val=1.0, diag=True)
            mask = const_pool.tile([128, 128], BF16)
            nc.vector.tensor_copy(mask, mask32)
            # ---- persistent SBUF ----
            qt = [persist.tile([128, T], BF16, tag=f"qt{i}", name=f"qt{i}")
                  for i in range(4)]
            kdup = [persist.tile([128, T], BF16, tag=f"kd{g}", name=f"kd{g}")
                    for g in range(NG_LOC)]
            v_sb = [persist.tile([128, T], BF16, tag=f"v{g}", name=f"v{g}")
                    for g in range(NG_LOC)]
            xts = [persist.tile([128, T], BF16, tag=f"x{ct}", name=f"x{ct}")
                   for ct in range(NCT)]
            wq_sb = [persist.tile([128, S], BF16, tag=f"wq{ct}", name=f"wq{ct}")
                     for ct in range(NCT)]
            wk_sb = [persist.tile([128, 128], BF16, tag=f"wk{ct}", name=f"wk{ct}")
                     for ct in range(NCT)]
            wv_sb = [persist.tile([128, 128], BF16, tag=f"wv{ct}", name=f"wv{ct}")
                     for ct in range(NCT)]
            wp_sb = [persist.tile([128, C], BF16, tag=f"wp{i}", name=f"wp{i}")
                     for i in range(4)]

            # ---- DMAs: block-0 x first (unblocks A0), weights, rest of x ----
            for ct in range(NCT):
                nc.sync.dma_start(out=xts[ct][:, 0:512],
                                  in_=xT[ct * 128:(ct + 1) * 128, 0:512])
            for ct in range(NCT):
                nc.sync.dma_start(out=wq_sb[ct], in_=wqT[ct * 128:(ct + 1) * 128, :])
                nc.sync.dma_start(out=wk_sb[ct], in_=wkT[ct * 128:(ct + 1) * 128, :])
                nc.sync.dma_start(out=wv_sb[ct], in_=wvT[ct * 128:(ct + 1) * 128, :])
            for ct in range(NCT):
                nc.sync.dma_start(out=xts[ct][:, 512:1024],
                                  in_=xT[ct * 128:(ct + 1) * 128, 512:1024])
            for ct in range(NCT):
                nc.sync.dma_start(out=xts[ct][:, 1024:2048],
                                  in_=xT[ct * 128:(ct + 1) * 128, 1024:2048])
            for i in range(4):
                nc.sync.dma_start(out=wp_sb[i], in_=wpT[i * 128:(i + 1) * 128, :])

            # ones columns of v_sb (denominator trick)
            ones64 = const_pool.tile([128, 64], F32)
            nc.vector.memset(ones64, 1.0)
            for g in range(NG_LOC):
                for t in range(T // 128):
                    nc.vector.tensor_copy(
                        v_sb[g][:, t * 128:t * 128 + 64], ones64)

            # ---- generators for interleavable PE work ----
            def proj_block(j, use_s_pool):
                """Projections q/k/v for tq/tk block j + v transpose."""
                cols = slice(j * TQB, (j + 1) * TQB)

                def fresh():
                    if use_s_pool:
                        psx = psS.tile([128, 2 * TQB], F32, tag="s", name="psx")
                        return psx[:, 0:TQB]
                    return psM.tile([128, TQB], F32, tag="mm", name="psm")

                for p4 in range(4):
                    dst = fresh()
                    for ct in range(NCT):
                        nc.tensor.matmul(
                            dst, wq_sb[ct][:, p4 * 128:(p4 + 1) * 128],
                            xts[ct][:, cols], start=(ct == 0), stop=(ct == NCT - 1))
                        yield
                    nc.vector.tensor_copy(qt[p4][:, cols], dst)
                # k (both groups in one psum: g0 on 0:64, g1 on 64:128)
                dst = fresh()
                for ct in range(NCT):
                    nc.tensor.matmul(dst, wk_sb[ct], xts[ct][:, cols],
                                     start=(ct == 0), stop=(ct == NCT - 1))
                    yield
                for g in range(NG_LOC):
                    rows = slice(g * 64, (g + 1) * 64)
                    nc.vector.tensor_copy(kdup[g][0:64, cols], dst[rows, :])
                    nc.vector.tensor_copy(kdup[g][64:128, cols], dst[rows, :])
                # v -> vt (sbuf) -> per-128-block transpose into v_sb
                dst = fresh()
                for ct in range(NCT):
                    nc.tensor.matmul(dst, wv_sb[ct], xts[ct][:, cols],
                                     start=(ct == 0), stop=(ct == NCT - 1))
                    yield
                vt = vtp.tile([128, TQB], F32, tag="vt", name="vt")
                nc.vector.tensor_copy(vt, dst)
                for g in range(NG_LOC):
                    for ts_ in range(4):
                        t_abs = 4 * j + ts_
                        pst = psT.tile([128, 512], F32, tag="tr", name="pst")
                        nc.tensor.transpose(
                            pst[:, 0:64],
                            vt[g * 64:(g + 1) * 64, ts_ * 128:(ts_ + 1) * 128],
                            ident[g * 64:(g + 1) * 64, 0:64])
                        yield
                        nc.vector.tensor_copy(
                            v_sb[g][:, t_abs * 128 + 64:(t_abs + 1) * 128],
                            pst[:, 0:64])

            def outproj_block(j, at_tiles):
                """Output projection for tq block j (4 tau rows of 128)."""
                for tt in range(4):
                    tau = 4 * j + tt
                    ysb = ypool.tile([128, C], BF16, tag="y", name="ysb")
                    for half in range(2):
                        if (tt * 2 + half) % 2 == 0:
                            yp = psM.tile([128, TQB], F32, tag="mm", name="yp")
                        else:
                            yp = psT.tile([128, TQB], F32, tag="tr", name="yp")
                        for p4 in range(4):
                            nc.tensor.matmul(
                                yp, at_tiles[p4][:, tt * 128:(tt + 1) * 128],
                                wp_sb[p4][:, half * TQB:(half + 1) * TQB],
                                start=(p4 == 0), stop=(p4 == 3))
                            yield
                        cols = slice(half * TQB, (half + 1) * TQB)
                        nc.vector.tensor_copy(ysb[:, cols], yp)
                        nc.sync.dma_start(
                            out=y[tau * 128:(tau + 1) * 128, cols],
                            in_=ysb[:, cols])

            # ---- attention ----
            def emit_ep(j, p4, t, ps, po, ntk, rcp_tile=None):
                """exp + mask + PV pair for step t of pair p4, block j."""
                g = p4 // 2
                c = t - 4 * j
                off = max(0, c * 128)
                pt = ppool.tile([128, 2 * TQB], BF16, tag="pt", name="pt")
                if off == 0:
                    # both heads' regions are contiguous: one 1024-col exp
                    nc.scalar.activation(pt[:, :], ps[:, :], EXP, scale=SCALE)
                else:
                    nc.scalar.activation(pt[:, off:TQB], ps[:, off:TQB],
                                         EXP, scale=SCALE)
                    nc.scalar.activation(pt[:, TQB + off:2 * TQB],
                                         ps[:, TQB + off:2 * TQB],
                                         EXP, scale=SCALE)
                if c >= 0:
                    nc.gpsimd.tensor_mul(
                        pt[:, off:off + 128], pt[:, off:off + 128], mask)
                    nc.gpsimd.tensor_mul(
                        pt[:, TQB + off:TQB + off + 128],
                        pt[:, TQB + off:TQB + off + 128], mask)
                for h01 in range(2):
                    nc.tensor.matmul(
                        po[:, h01 * TQB + off:(h01 + 1) * TQB],
                        v_sb[g][:, t * 128:(t + 1) * 128],
                        pt[:, h01 * TQB + off:(h01 + 1) * TQB],
                        start=(t == 0), stop=(t == ntk - 1))
                    if t == ntk - 1 and rcp_tile is not None:
                        nc.vector.reciprocal_approx_fast(
                            rcp_tile[0:64, h01 * TQB:(h01 + 1) * TQB],
                            po[0:64, h01 * TQB:(h01 + 1) * TQB])

            def attention_block(j, feed, rate):
                """Attention for tq block j; drains `feed` generators at
                ~`rate` PE ops per step."""
                tq0 = j * TQB
                ntk = 4 * (j + 1)
                at_tiles = [apool.tile([128, TQB], BF16, tag=f"at{p4}",
                                       name=f"at{j}_{p4}")
                            for p4 in range(4)]
                budget = 0.0
                for p4 in range(4):
                    g = p4 // 2
                    po = psO.tile([128, 2 * TQB], F32, tag="po", name="po")
                    ps_prev = None
                    for t in range(ntk):
                        c = t - 4 * j
                        off = max(0, c * 128)
                        ps = psS.tile([128, 2 * TQB], F32, tag="s", name="ps")
                        nc.tensor.matmul(
                            ps[:, off:TQB],
                            kdup[g][0:64, t * 128:(t + 1) * 128],
                            qt[p4][0:64, tq0 + off:tq0 + TQB],
                            start=True, stop=True)
                        nc.tensor.matmul(
                            ps[:, TQB + off:2 * TQB],
                            kdup[g][64:128, t * 128:(t + 1) * 128],
                            qt[p4][64:128, tq0 + off:tq0 + TQB],
                            start=True, stop=True)
                        if t > 0:
                            emit_ep(j, p4, t - 1, ps_prev, po, ntk)
                        ps_prev = ps
                        budget += rate
                        while budget >= 1.0 and feed:
                            try:
                                next(feed[0])
                                budget -= 1.0
                            except StopIteration:
                                feed.pop(0)
                    rcp = small.tile([128, 2 * TQB], F32, tag="rcp", name="rcp")
                    emit_ep(j, p4, ntk - 1, ps_prev, po, ntk, rcp_tile=rcp)
                    # normalization (recips were emitted inside emit_ep)
                    nc.vector.tensor_mul(
                        at_tiles[p4][0:64, :], po[64:128, 0:TQB],
                        rcp[0:64, 0:TQB])
                    nc.vector.tensor_mul(
                        at_tiles[p4][64:128, :], po[64:128, TQB:2 * TQB],
                        rcp[0:64, TQB:2 * TQB])
                return at_tiles

            def drain(gen):
                for _ in gen:
                    pass

            # ---- schedule ----
            drain(proj_block(0, use_s_pool=True))
            feed = [proj_block(1, use_s_pool=False),
                    proj_block(2, use_s_pool=False),
                    proj_block(3, use_s_pool=False)]
            at0 = attention_block(0, feed, 3.8)
            feed.append(outproj_block(0, at0))
            at1 = attention_block(1, feed, 1.45)
            feed.append(outproj_block(1, at1))
            at2 = attention_block(2, feed, 1.3)
            feed.append(outproj_block(2, at2))
            at3 = attention_block(3, feed, 1.2)
            for gen in feed:
                drain(gen)
            drain(outproj_block(3, at3))

    nc.compile()
    return nc


_NC_CACHE = None


def _get_nc():
    global _NC_CACHE
    if _NC_CACHE is None:
        _NC_CACHE = _build_program()
    return _NC_CACHE


def _bf16(a):
    return np.ascontiguousarray(a).astype(ml_dtypes.bfloat16)


def _make_in_maps(x, Wq, Wk, Wv, Wp):
    in_maps = []
    for core in range(8):
        b, tp = core // 2, core % 2
        hs = slice(tp * NH_LOC, (tp + 1) * NH_LOC)
        gs = slice(tp * NG_LOC, (tp + 1) * NG_LOC)
        in_maps.append({
            "xT": _bf16(x[b].T),
            "wqT": _bf16(Wq[hs].transpose(2, 0, 1).reshape(C, S)),
            "wkT": _bf16(Wk[gs].transpose(2, 0, 1).reshape(C, NG_LOC * HD)),
            "wvT": _bf16(Wv[gs].transpose(2, 0, 1).reshape(C, NG_LOC * HD)),
            "wpT": _bf16(Wp[:, tp * S:(tp + 1) * S].T),
        })
    return in_maps


def kernel(x, Wq, Wk, Wv, Wp, bp, _trace=False):
    x = np.asarray(x, dtype=np.float32)
    nc = _get_nc()
    in_maps = _make_in_maps(
        x, np.asarray(Wq, np.float32), np.asarray(Wk, np.float32),
        np.asarray(Wv, np.float32), np.asarray(Wp, np.float32))
    res = run_bass_kernel_spmd(nc, in_maps, list(range(8)), trace=_trace)
    out = np.empty((B, T, C), dtype=np.float32)
    bp32 = np.asarray(bp, np.float32)
    for b in range(B):
        out[b] = (res.results[2 * b]["y"].astype(np.float32)
                  + res.results[2 * b + 1]["y"].astype(np.float32) + bp32)
    if _trace:
        return out, res
    return out


# revision 9
# speedup vs baseline: 1.1079x; 1.0226x over previous
"""GroupQueryAttention TRN2 Bass kernel, v2.

Problem: B=4, T=2048, C=1024, H=16 heads, G=4 groups, head_dim=64, causal.
Sharding: 8 cores = 4 batches (DP) x 2 tensor-parallel halves (8 heads /
2 groups each). Host pre-transposes x and weight slices to bf16; each core
computes a partial output projection over its 512 attention channels; host
sums the two TP partials per batch and adds the bias.

v2 design (vs v1 at ~329us):
- all-bf16 datapath (halves DMA bytes; bf16 matmuls stream 1 col/cycle and
  get FWL, fp32r measured ~1.5 cyc/col)
- head pairs (2p4, 2p4+1) share one [128,2,512] psum score tile (2 banks)
  so ONE 1024-col exp ACTIVATE serves both heads: the ACT engine is the
  bottleneck (139k exp columns + ~300 cyc/instruction overhead) and halving
  the instruction count cuts its overhead in half
- skew-1 software pipelining in the attention inner loop: scores for step t
  are emitted before PV for step t-1, so the PE never head-of-line blocks
  on the ACT exp (v1 lost ~30% PE occupancy to this)
- score matmul pairs are emitted back-to-back on row bands 0:64 / 64:128
  (tile_position row groups) so the PE can overlap them
- ~20 warmup matmuls on a memset tile at t=0: HAM clock-gate releases only
  after ~3.4us of sustained PE activity (v1 ran its first 53us at 1.2 GHz)
- phase fusion: projections for block j+2 and output projections for
  blocks j-2/j-1 are interleaved into attention block j's steps, keeping
  the ACT exp stream and the PE both busy end-to-end
"""

import sys
import numpy as np
import ml_dtypes

for _p in ("/opt/trn_rl_repo", "/opt/trn_rl_repo/concourse"):
    if _p not in sys.path:
        sys.path.insert(0, _p)

import concourse.bass as bass  # noqa: E402
import concourse.mybir as mybir  # noqa: E402
from concourse import bacc  # noqa: E402
from concourse.tile import TileContext  # noqa: E402
from concourse.bass_utils import run_bass_kernel_spmd  # noqa: E402
from concourse.masks import make_identity, make_upper_triangular  # noqa: E402

F32 = mybir.dt.float32
BF16 = mybir.dt.bfloat16
EXP = mybir.ActivationFunctionType.Exp

B, T, C = 4, 2048, 1024
NH, NG, HD = 16, 4, 64
NH_LOC, NG_LOC = 8, 2          # per-core heads / groups
S = NH_LOC * HD                # 512 local attention channels
TQB = 512                      # tq block
NTQB = T // TQB                # 4
NCT = C // 128                 # 8 contraction tiles
SCALE = float(HD) ** -0.5
N_WARMUP = 12


def _build_program():
    nc = bacc.Bacc("TRN2", target_bir_lowering=False, debug=False, num_devices=8)

    xT = nc.dram_tensor("xT", [C, T], BF16, kind="ExternalInput")
    wqT = nc.dram_tensor("wqT", [C, S], BF16, kind="ExternalInput")
    wkT = nc.dram_tensor("wkT", [C, NG_LOC * HD], BF16, kind="ExternalInput")
    wvT = nc.dram_tensor("wvT", [C, NG_LOC * HD], BF16, kind="ExternalInput")
    wpT = nc.dram_tensor("wpT", [S, C], BF16, kind="ExternalInput")
    y = nc.dram_tensor("y", [T, C], BF16, kind="ExternalOutput")

    with TileContext(nc) as tc:
        with tc.tile_pool(name="const", bufs=1) as const_pool, \
             tc.tile_pool(name="persist", bufs=1) as persist, \
             tc.tile_pool(name="vtp", bufs=2) as vtp, \
             tc.tile_pool(name="pp", bufs=3) as ppool, \
             tc.tile_pool(name="attn", bufs=4) as apool, \
             tc.tile_pool(name="sm", bufs=2) as small, \
             tc.tile_pool(name="yo", bufs=3) as ypool, \
             tc.tile_pool(name="psS", bufs=2, space="PSUM") as psS, \
             tc.tile_pool(name="psO", bufs=1, space="PSUM") as psO, \
             tc.tile_pool(name="psM", bufs=1, space="PSUM") as psM, \
             tc.tile_pool(name="psT", bufs=1, space="PSUM") as psT:

            # ---- warmup first: PE busy from t~0 releases the HAM gate ----
            wtile = const_pool.tile([128, 512], BF16)
            nc.gpsimd.memset(wtile, 0.125)
            for _ in range(N_WARMUP):
                pswu = psM.tile([128, 512], F32, tag="mm", name="pswu")
                nc.tensor.matmul(pswu, wtile[:, 0:128], wtile,
                                 start=True, stop=True)

            # ---- constants ----
            ident = const_pool.tile([128, 64], F32)
            make_identity(nc, ident[0:64, 0:64])
            make_identity(nc, ident[64:128, 0:64], nomemset=False)
            mask32 = const_pool.tile([128, 128], F32)
            make_upper_triangular(nc, mask32, val=1.0, diag=True)
            mask = const_pool.tile([128, 128], BF16)
            nc.vector.tensor_copy(mask, mask32)
            # ---- persistent SBUF ----
            qt = [persist.tile([128, T], BF16, tag=f"qt{i}", name=f"qt{i}")
                  for i in range(4)]
            kdup = [persist.tile([128, T], BF16, tag=f"kd{g}", name=f"kd{g}")
                    for g in range(NG_LOC)]
            v_sb = [persist.tile([128, T], BF16, tag=f"v{g}", name=f"v{g}")
                    for g in range(NG_LOC)]
            xts = [persist.tile([128, T], BF16, tag=f"x{ct}", name=f"x{ct}")
                   for ct in range(NCT)]
            wq_sb = [persist.tile([128, S], BF16, tag=f"wq{ct}", name=f"wq{ct}")
                     for ct in range(NCT)]
            wk_sb = [persist.tile([128, 128], BF16, tag=f"wk{ct}", name=f"wk{ct}")
                     for ct in range(NCT)]
            wv_sb = [persist.tile([128, 128], BF16, tag=f"wv{ct}", name=f"wv{ct}")
                     for ct in range(NCT)]
            wp_sb = [persist.tile([128, C], BF16, tag=f"wp{i}", name=f"wp{i}")
                     for i in range(4)]

            # ---- DMAs: block-0 x first (unblocks A0), weights, rest of x ----
            for ct in range(NCT):
                nc.sync.dma_start(out=xts[ct][:, 0:512],
                                  in_=xT[ct * 128:(ct + 1) * 128, 0:512])
            for ct in range(NCT):
                nc.sync.dma_start(out=wq_sb[ct], in_=wqT[ct * 128:(ct + 1) * 128, :])
                nc.sync.dma_start(out=wk_sb[ct], in_=wkT[ct * 128:(ct + 1) * 128, :])
                nc.sync.dma_start(out=wv_sb[ct], in_=wvT[ct * 128:(ct + 1) * 128, :])
            for ct in range(NCT):
                nc.sync.dma_start(out=xts[ct][:, 512:1024],
                                  in_=xT[ct * 128:(ct + 1) * 128, 512:1024])
            for ct in range(NCT):
                nc.sync.dma_start(out=xts[ct][:, 1024:2048],
                                  in_=xT[ct * 128:(ct + 1) * 128, 1024:2048])
            for i in range(4):
                nc.sync.dma_start(out=wp_sb[i], in_=wpT[i * 128:(i + 1) * 128, :])

            # ones columns of v_sb (denominator trick)
            ones64 = const_pool.tile([128, 64], F32)
            nc.vector.memset(ones64, 1.0)
            for g in range(NG_LOC):
                for t in range(T // 128):
                    nc.vector.tensor_copy(
                        v_sb[g][:, t * 128:t * 128 + 64], ones64)

            # ---- generators for interleavable PE work ----
            def proj_block(j, use_s_pool):
                """Projections q/k/v for tq/tk block j + v transpose."""
                cols = slice(j * TQB, (j + 1) * TQB)

                def fresh():
                    if use_s_pool:
                        psx = psS.tile([128, 2 * TQB], F32, tag="s", name="psx")
                        return psx[:, 0:TQB]
                    return psM.tile([128, TQB], F32, tag="mm", name="psm")

                for p4 in range(4):
                    dst = fresh()
                    for ct in range(NCT):
                        nc.tensor.matmul(
                            dst, wq_sb[ct][:, p4 * 128:(p4 + 1) * 128],
                            xts[ct][:, cols], start=(ct == 0), stop=(ct == NCT - 1))
                        yield
                    nc.vector.tensor_copy(qt[p4][:, cols], dst)
                # k (both groups in one psum: g0 on 0:64, g1 on 64:128)
                dst = fresh()
                for ct in range(NCT):
                    nc.tensor.matmul(dst, wk_sb[ct], xts[ct][:, cols],
                                     start=(ct == 0), stop=(ct == NCT - 1))
                    yield
                for g in range(NG_LOC):
                    rows = slice(g * 64, (g + 1) * 64)
                    nc.vector.tensor_copy(kdup[g][0:64, cols], dst[rows, :])
                    nc.vector.tensor_copy(kdup[g][64:128, cols], dst[rows, :])
                # v -> vt (sbuf) -> per-128-block transpose into v_sb
                dst = fresh()
                for ct in range(NCT):
                    nc.tensor.matmul(dst, wv_sb[ct], xts[ct][:, cols],
                                     start=(ct == 0), stop=(ct == NCT - 1))
                    yield
                vt = vtp.tile([128, TQB], F32, tag="vt", name="vt")
                nc.vector.tensor_copy(vt, dst)
                for g in range(NG_LOC):
                    for ts_ in range(4):
                        t_abs = 4 * j + ts_
                        pst = psT.tile([128, 512], F32, tag="tr", name="pst")
                        nc.tensor.transpose(
                            pst[:, 0:64],
                            vt[g * 64:(g + 1) * 64, ts_ * 128:(ts_ + 1) * 128],
                            ident[g * 64:(g + 1) * 64, 0:64])
                        yield
                        nc.vector.tensor_copy(
                            v_sb[g][:, t_abs * 128 + 64:(t_abs + 1) * 128],
                            pst[:, 0:64])

            def outproj_block(j, at_tiles):
                """Output projection for tq block j (4 tau rows of 128)."""
                for tt in range(4):
                    tau = 4 * j + tt
                    ysb = ypool.tile([128, C], BF16, tag="y", name="ysb")
                    for half in range(2):
                        if (tt * 2 + half) % 2 == 0:
                            yp = psM.tile([128, TQB], F32, tag="mm", name="yp")
                        else:
                            yp = psT.tile([128, TQB], F32, tag="tr", name="yp")
                        for p4 in range(4):
                            nc.tensor.matmul(
                                yp, at_tiles[p4][:, tt * 128:(tt + 1) * 128],
                                wp_sb[p4][:, half * TQB:(half + 1) * TQB],
                                start=(p4 == 0), stop=(p4 == 3))
                            yield
                        cols = slice(half * TQB, (half + 1) * TQB)
                        nc.vector.tensor_copy(ysb[:, cols], yp)
                        nc.sync.dma_start(
                            out=y[tau * 128:(tau + 1) * 128, cols],
                            in_=ysb[:, cols])

            # ---- attention ----
            def emit_ep(j, p4, t, ps, po, ntk, rcp_tile=None):
                """exp + mask + PV pair for step t of pair p4, block j."""
                g = p4 // 2
                c = t - 4 * j
                off = max(0, c * 128)
                pt = ppool.tile([128, 2 * TQB], BF16, tag="pt", name="pt")
                if off == 0:
                    # both heads' regions are contiguous: one 1024-col exp
                    nc.scalar.activation(pt[:, :], ps[:, :], EXP, scale=SCALE)
                else:
                    nc.scalar.activation(pt[:, off:TQB], ps[:, off:TQB],
                                         EXP, scale=SCALE)
                    nc.scalar.activation(pt[:, TQB + off:2 * TQB],
                                         ps[:, TQB + off:2 * TQB],
                                         EXP, scale=SCALE)
                if c >= 0:
                    nc.vector.tensor_mul(
                        pt[:, off:off + 128], pt[:, off:off + 128], mask)
                    nc.gpsimd.tensor_mul(
                        pt[:, TQB + off:TQB + off + 128],
                        pt[:, TQB + off:TQB + off + 128], mask)
                for h01 in range(2):
                    nc.tensor.matmul(
                        po[:, h01 * TQB + off:(h01 + 1) * TQB],
                        v_sb[g][:, t * 128:(t + 1) * 128],
                        pt[:, h01 * TQB + off:(h01 + 1) * TQB],
                        start=(t == 0), stop=(t == ntk - 1))
                    if t == ntk - 1 and rcp_tile is not None:
                        nc.vector.reciprocal_approx_fast(
                            rcp_tile[0:64, h01 * TQB:(h01 + 1) * TQB],
                            po[0:64, h01 * TQB:(h01 + 1) * TQB])

            def attention_block(j, feed, rate):
                """Attention for tq block j; drains `feed` generators at
                ~`rate` PE ops per step."""
                tq0 = j * TQB
                ntk = 4 * (j + 1)
                at_tiles = [apool.tile([128, TQB], BF16, tag=f"at{p4}",
                                       name=f"at{j}_{p4}")
                            for p4 in range(4)]
                budget = 0.0
                for p4 in range(4):
                    g = p4 // 2
                    po = psO.tile([128, 2 * TQB], F32, tag="po", name="po")
                    ps_prev = None
                    for t in range(ntk):
                        c = t - 4 * j
                        off = max(0, c * 128)
                        ps = psS.tile([128, 2 * TQB], F32, tag="s", name="ps")
                        nc.tensor.matmul(
                            ps[:, off:TQB],
                            kdup[g][0:64, t * 128:(t + 1) * 128],
                            qt[p4][0:64, tq0 + off:tq0 + TQB],
                            start=True, stop=True)
                        nc.tensor.matmul(
                            ps[:, TQB + off:2 * TQB],
                            kdup[g][64:128, t * 128:(t + 1) * 128],
                            qt[p4][64:128, tq0 + off:tq0 + TQB],
                            start=True, stop=True)
                        if t > 0:
                            emit_ep(j, p4, t - 1, ps_prev, po, ntk)
                        ps_prev = ps
                        budget += rate
                        while budget >= 1.0 and feed:
                            try:
                                next(feed[0])
                                budget -= 1.0
                            except StopIteration:
                                feed.pop(0)
                    rcp = small.tile([128, 2 * TQB], F32, tag="rcp", name="rcp")
                    emit_ep(j, p4, ntk - 1, ps_prev, po, ntk, rcp_tile=rcp)
                    # normalization (recips were emitted inside emit_ep)
                    nc.vector.tensor_mul(
                        at_tiles[p4][0:64, :], po[64:128, 0:TQB],
                        rcp[0:64, 0:TQB])
                    nc.vector.tensor_mul(
                        at_tiles[p4][64:128, :], po[64:128, TQB:2 * TQB],
                        rcp[0:64, TQB:2 * TQB])
                return at_tiles

            def drain(gen):
                for _ in gen:
                    pass

            # ---- schedule ----
            drain(proj_block(0, use_s_pool=True))
            feed = [proj_block(1, use_s_pool=False),
                    proj_block(2, use_s_pool=False),
                    proj_block(3, use_s_pool=False)]
            at0 = attention_block(0, feed, 3.8)
            feed.append(outproj_block(0, at0))
            at1 = attention_block(1, feed, 1.45)
            feed.append(outproj_block(1, at1))
            at2 = attention_block(2, feed, 1.3)
            feed.append(outproj_block(2, at2))
            at3 = attention_block(3, feed, 1.2)
            for gen in feed:
                drain(gen)
            drain(outproj_block(3, at3))

    nc.compile()
    return nc


_NC_CACHE = None


def _get_nc():
    global _NC_CACHE
    if _NC_CACHE is None:
        _NC_CACHE = _build_program()
    return _NC_CACHE


def _bf16(a):
    return np.ascontiguousarray(a).astype(ml_dtypes.bfloat16)


def _make_in_maps(x, Wq, Wk, Wv, Wp):
    in_maps = []
    for core in range(8):
        b, tp = core // 2, core % 2
        hs = slice(tp * NH_LOC, (tp + 1) * NH_LOC)
        gs = slice(tp * NG_LOC, (tp + 1) * NG_LOC)
        in_maps.append({
            "xT": _bf16(x[b].T),
            "wqT": _bf16(Wq[hs].transpose(2, 0, 1).reshape(C, S)),
            "wkT": _bf16(Wk[gs].transpose(2, 0, 1).reshape(C, NG_LOC * HD)),
            "wvT": _bf16(Wv[gs].transpose(2, 0, 1).reshape(C, NG_LOC * HD)),
            "wpT": _bf16(Wp[:, tp * S:(tp + 1) * S].T),
        })
    return in_maps


def kernel(x, Wq, Wk, Wv, Wp, bp, _trace=False):
    x = np.asarray(x, dtype=np.float32)
    nc = _get_nc()
    in_maps = _make_in_maps(
        x, np.asarray(Wq, np.float32), np.asarray(Wk, np.float32),
        np.asarray(Wv, np.float32), np.asarray(Wp, np.float32))
    res = run_bass_kernel_spmd(nc, in_maps, list(range(8)), trace=_trace)
    out = np.empty((B, T, C), dtype=np.float32)
    bp32 = np.asarray(bp, np.float32)
    for b in range(B):
        out[b] = (res.results[2 * b]["y"].astype(np.float32)
                  + res.results[2 * b + 1]["y"].astype(np.float32) + bp32)
    if _trace:
        return out, res
    return out
